# revision 1
# baseline (speedup 1.0000x reference)
"""Kimi-style MoE (8 routed experts top-2 + shared expert) on 8 Trainium2 cores.

Sharding: expert-parallel. Core c owns routed expert c (dense over all T tokens,
combine-weighted on device) plus a 1/8 intermediate-dim shard of the shared
expert. The gate (fp32) is replicated on every core; each core extracts its own
expert's combine column with a one-hot selector so the program is pure SPMD.
Each core returns a partial-sum [D, T] output; the host sums the 8 partials.

All expert matmuls run in bf16 (fp32 PSUM accumulation); the gate runs in fp32
because top-k selection is precision-critical.
"""

import sys

for _p in ("/opt/trn_rl_repo", "/opt/pypackages"):
    if _p not in sys.path:
        sys.path.insert(0, _p)

import numpy as np
import ml_dtypes

import concourse.bass as bass
import concourse.mybir as mybir
import concourse.tile as tile
from concourse import bacc
from concourse.bass import ts
from concourse.bass_utils import run_bass_kernel_spmd
from concourse.masks import make_identity

BF16 = mybir.dt.bfloat16
F32 = mybir.dt.float32
NP_BF16 = ml_dtypes.bfloat16

# Problem shapes (hardcoded per the contract).
B, S, D = 2, 1024, 1024
E, TOPK = 8, 2
I = 1408
N_SHARED = 2
I_SH = N_SHARED * I          # 2816
SCALE = 2.5
T = B * S                    # 2048
P = 128
NT = T // 512                # 4 free-dim tiles of 512 tokens
KO = D // P                  # 8 contraction subtiles
JR = I // P                  # 11 routed (v,g) pair tiles
JS_TOT = I_SH // P           # 22 shared pair tiles over all cores
JS = 3                       # shared pair tiles per core (padded)
KD = JR + JS                 # 14 down-proj contraction tiles
DT = D // P                  # 8 output partition tiles
N_CORES = 8

BIG = 1.0e9


def _body(tc, io, uid=0):
    nc = tc.nc

    with (
        tc.tile_pool(name="const", bufs=1) as cpool,
        tc.tile_pool(name="w1s", bufs=3) as w1pool,
        tc.tile_pool(name="sv", bufs=4) as svpool,
        tc.tile_pool(name="outs", bufs=4) as opool,
    ):
        # ---- resident SBUF tensors ----
        xT = cpool.tile([P, KO, T], BF16, tag="xT")
        wd = cpool.tile([P, KD, DT, P], BF16, tag="wd")
        gw = cpool.tile([P, KO, E], F32, tag="gw")
        gb = cpool.tile([P, E], F32, tag="gb")
        sel = cpool.tile([P, E], F32, tag="sel")
        b1 = cpool.tile([P, 2 * JR], F32, tag="b1")
        bs1 = cpool.tile([P, 2 * JS], F32, tag="bs1")
        b2 = cpool.tile([P, DT], F32, tag="b2")
        bs2 = cpool.tile([P, DT], F32, tag="bs2")
        h_all = cpool.tile([P, KD, T], BF16, tag="h_all")
        w_bcast = cpool.tile([P, T], F32, tag="w_bcast")
        ident = cpool.tile([P, P], F32, tag="ident")
        ones1 = cpool.tile([1, P], F32, tag="ones1")

        for k in range(KO):
            nc.sync.dma_start(xT[:, k], io["xT"][:, k])
        for kd in range(KD):
            nc.sync.dma_start(wd[:, kd], io["wd"][:, kd])
        nc.sync.dma_start(gw[:], io["gwT"][:])
        nc.sync.dma_start(gb[:], io["gbias"][:])
        nc.sync.dma_start(sel[:], io["sel"][:])
        nc.sync.dma_start(b1[:], io["b1t"][:])
        nc.sync.dma_start(bs1[:], io["bs1t"][:])
        nc.sync.dma_start(b2[:], io["b2c"][:])
        nc.sync.dma_start(bs2[:], io["bs2c"][:])
        make_identity(nc, ident[:])
        nc.vector.memset(ones1[:], 1.0)

        # ---- gate: logits [T,8] in fp32, token tiles on partitions ----
        s_all = cpool.tile([P, T // P, E], F32, tag="s_all")
        with (
            tc.tile_pool(name="gpsum", bufs=2, space="PSUM") as gpsum,
            tc.tile_pool(name="gx", bufs=3) as gxpool,
        ):
            for mt in range(T // P):
                xg = gxpool.tile([P, KO, P], F32, tag="xg")
                nc.sync.dma_start(xg[:], io["xT32"][:, :, ts(mt, P)])
                pg = gpsum.tile([P, E], F32, tag="pg")
                for k in range(KO):
                    nc.tensor.matmul(
                        pg[:],
                        xg[:, k],
                        gw[:, k],
                        start=(k == 0),
                        stop=(k == KO - 1),
                    )
                # scores = sigmoid(logits)
                nc.scalar.activation(
                    s_all[:, mt], pg[:], mybir.ActivationFunctionType.Sigmoid
                )

        MT = T // P
        gtmp = cpool.tile([P, MT, E], F32, tag="gtmp")
        gtmp2 = cpool.tile([P, MT, E], F32, tag="gtmp2")
        m1 = cpool.tile([P, MT], F32, tag="m1")
        m2 = cpool.tile([P, MT], F32, tag="m2")
        wq = cpool.tile([P, MT], F32, tag="wq")
        add = mybir.AluOpType.add
        mult = mybir.AluOpType.mult
        # s += gate_bias (broadcast over token tiles)
        nc.vector.tensor_tensor(
            s_all[:], s_all[:], gb[:, None, :].to_broadcast((P, MT, E)), add
        )
        nc.vector.reduce_max(m1[:], s_all[:], axis=mybir.AxisListType.X)
        nc.vector.tensor_tensor(
            gtmp[:], s_all[:], m1[:, :, None].to_broadcast((P, MT, E)),
            mybir.AluOpType.is_equal,
        )
        # s2 = s - BIG * eq1
        nc.vector.scalar_tensor_tensor(
            gtmp2[:], gtmp[:], -BIG, s_all[:], mult, add
        )
        nc.vector.reduce_max(m2[:], gtmp2[:], axis=mybir.AxisListType.X)
        # mask = eq1 + eq2  (gtmp <- mask)
        nc.vector.tensor_tensor(
            gtmp2[:], gtmp2[:], m2[:, :, None].to_broadcast((P, MT, E)),
            mybir.AluOpType.is_equal,
        )
        nc.vector.tensor_tensor(gtmp[:], gtmp[:], gtmp2[:], add)
        # wq = SCALE * sum(s * mask * sel) / (m1 + m2)
        nc.vector.tensor_tensor(gtmp[:], gtmp[:], s_all[:], mult)
        nc.vector.tensor_tensor(
            gtmp[:], gtmp[:], sel[:, None, :].to_broadcast((P, MT, E)), mult
        )
        nc.vector.reduce_sum(wq[:], gtmp[:], axis=mybir.AxisListType.X)
        nc.vector.tensor_tensor(m1[:], m1[:], m2[:], add)
        nc.vector.reciprocal(m2[:], m1[:])
        nc.vector.tensor_scalar_mul(m2[:], m2[:], SCALE)
        nc.vector.tensor_tensor(wq[:], wq[:], m2[:], mult)

        # ---- broadcast wq [tokens on partitions] -> w_bcast [P, T] ----
        w_t = cpool.tile([P, P], F32, tag="w_t")
        w_row = cpool.tile([1, T], F32, tag="w_row")
        wrow_dram = nc.dram_tensor(f"wrow_scratch_{uid}", [T], F32)
        with tc.tile_pool(name="tpsum", bufs=2, space="PSUM") as tpsum:
            pt = tpsum.tile([P, P], F32, tag="pt")
            nc.tensor.transpose(pt[:MT, :], wq[:], ident[:])
            nc.vector.tensor_copy(w_t[:MT, :], pt[:MT, :])
            nc.sync.dma_start(
                wrow_dram[:].rearrange("(p f) -> p f", p=MT), w_t[:MT, :]
            )
            nc.sync.dma_start(w_row[:], wrow_dram[None, :])
            for t in range(NT):
                pb = tpsum.tile([P, 512], F32, tag="pb")
                nc.tensor.matmul(
                    pb[:], ones1[:], w_row[:, ts(t, 512)], start=True, stop=True
                )
                nc.vector.tensor_copy(w_bcast[:, ts(t, 512)], pb[:])

        # ---- up projections + swiglu -> h_all ----
        # routed pairs j in [0, JR); shared pairs j in [JR, KD)
        with tc.tile_pool(name="upsum", bufs=4, space="PSUM") as upsum:
            for j in range(KD):
                routed = j < JR
                wsrc = io["w1t"] if routed else io["ws1t"]
                jj = j if routed else j - JR
                bsrc = b1 if routed else bs1
                w1tile = w1pool.tile([P, KO, 2 * P], BF16, tag="w1tile")
                nc.sync.dma_start(w1tile[:], wsrc[:, jj])
                for t in range(NT):
                    pv = upsum.tile([P, 512], F32, tag="pv")
                    pgu = upsum.tile([P, 512], F32, tag="pgu")
                    for k in range(KO):
                        nc.tensor.matmul(
                            pv[:], w1tile[:, k, :P], xT[:, k, ts(t, 512)],
                            start=(k == 0), stop=(k == KO - 1),
                        )
                    for k in range(KO):
                        nc.tensor.matmul(
                            pgu[:], w1tile[:, k, P:], xT[:, k, ts(t, 512)],
                            start=(k == 0), stop=(k == KO - 1),
                        )
                    sv = svpool.tile([P, 512], F32, tag="sv")
                    bias_v = bsrc[:, 2 * jj : 2 * jj + 1]
                    # sv = sigmoid(v + b1v)   (silu built from sigmoid so the
                    # numerics match jax's x*sigmoid(x) exactly)
                    nc.scalar.activation(
                        sv[:], pv[:], mybir.ActivationFunctionType.Sigmoid,
                        bias=bias_v,
                    )
                    # sv = (v + b1v) * sigmoid(v + b1v) = silu(v + b1v)
                    nc.vector.scalar_tensor_tensor(
                        sv[:], pv[:], bias_v, sv[:], add, mult
                    )
                    # h = (g + b1g) * sv
                    nc.vector.scalar_tensor_tensor(
                        h_all[:, j, ts(t, 512)], pgu[:],
                        bsrc[:, 2 * jj + 1 : 2 * jj + 2], sv[:], add, mult,
                    )

        # ---- down projection + bias/weight epilogue -> out ----
        # routed and shared accumulate in separate PSUM banks; the combine
        # weight applies to the routed result (incl. b2) at the output.
        with tc.tile_pool(name="dpsum", bufs=4, space="PSUM") as dpsum:
            for dt in range(DT):
                for t in range(NT):
                    pd_r = dpsum.tile([P, 512], F32, tag="pd_r")
                    pd_s = dpsum.tile([P, 512], F32, tag="pd_s")
                    for kd in range(JR):
                        nc.tensor.matmul(
                            pd_r[:], wd[:, kd, dt], h_all[:, kd, ts(t, 512)],
                            start=(kd == 0), stop=(kd == JR - 1),
                        )
                    for kd in range(JR, KD):
                        nc.tensor.matmul(
                            pd_s[:], wd[:, kd, dt], h_all[:, kd, ts(t, 512)],
                            start=(kd == JR), stop=(kd == KD - 1),
                        )
                    osb = opool.tile([P, 512], F32, tag="osb")
                    # osb = (pd_r + b2) * w
                    nc.vector.scalar_tensor_tensor(
                        osb[:], pd_r[:], b2[:, dt : dt + 1],
                        w_bcast[:, ts(t, 512)], add, mult,
                    )
                    # osb += pd_s + bs2  (bs2 zero on cores != 0)
                    nc.vector.scalar_tensor_tensor(
                        osb[:], pd_s[:], bs2[:, dt : dt + 1], osb[:], add, add,
                    )
                    nc.sync.dma_start(io["out"][ts(dt, P), ts(t, 512)], osb[:])


def build_nc(reps=1):
    nc = bacc.Bacc(None, target_bir_lowering=False, debug=False)
    io = {
        "xT": nc.declare_dram_parameter("xT", [P, KO, T], BF16, isOutput=False),
        "xT32": nc.declare_dram_parameter("xT32", [P, KO, T], F32, isOutput=False),
        "gwT": nc.declare_dram_parameter("gwT", [P, KO, E], F32, isOutput=False),
        "gbias": nc.declare_dram_parameter("gbias", [P, E], F32, isOutput=False),
        "sel": nc.declare_dram_parameter("sel", [P, E], F32, isOutput=False),
        "w1t": nc.declare_dram_parameter(
            "w1t", [P, JR, KO, 2 * P], BF16, isOutput=False
        ),
        "ws1t": nc.declare_dram_parameter(
            "ws1t", [P, JS, KO, 2 * P], BF16, isOutput=False
        ),
        "wd": nc.declare_dram_parameter("wd", [P, KD, DT, P], BF16, isOutput=False),
        "b1t": nc.declare_dram_parameter("b1t", [P, 2 * JR], F32, isOutput=False),
        "bs1t": nc.declare_dram_parameter("bs1t", [P, 2 * JS], F32, isOutput=False),
        "b2c": nc.declare_dram_parameter("b2c", [P, DT], F32, isOutput=False),
        "bs2c": nc.declare_dram_parameter("bs2c", [P, DT], F32, isOutput=False),
        "out": nc.declare_dram_parameter("out", [D, T], F32, isOutput=True),
    }
    with tile.TileContext(nc) as tc:
        for r in range(reps):
            _body(tc, io, uid=r)
    nc.compile()
    return nc


def _part_tiles(vec, n_tiles):
    """[n_tiles*128] -> [128, n_tiles] (partition-tiled per-row constants)."""
    return np.ascontiguousarray(vec.reshape(n_tiles, P).T.astype(np.float32))


def _shared_slices(core):
    """Global shared pair-tile indices owned by `core` (<= JS of them)."""
    counts = [3, 3, 3, 3, 3, 3, 2, 2]
    start = sum(counts[:core])
    return list(range(start, start + counts[core]))


def prep_inputs(inputs):
    """Full problem inputs -> list of 8 per-core in_maps (numpy arrays)."""
    x = np.asarray(inputs["x"], np.float32)
    gate_w = np.asarray(inputs["gate_w"], np.float32)
    gate_bias = np.asarray(inputs["gate_bias"], np.float32)
    W1 = np.asarray(inputs["W1"], np.float32)
    b1 = np.asarray(inputs["b1"], np.float32)
    W2 = np.asarray(inputs["W2"], np.float32)
    b2 = np.asarray(inputs["b2"], np.float32)
    Ws1 = np.asarray(inputs["Ws1"], np.float32)
    bs1 = np.asarray(inputs["bs1"], np.float32)
    Ws2 = np.asarray(inputs["Ws2"], np.float32)
    bs2 = np.asarray(inputs["bs2"], np.float32)

    xf = x.reshape(T, D)
    # xT_prep[p, ko, t] = xf[t, ko*128+p]
    xT32 = np.ascontiguousarray(xf.T.reshape(KO, P, T).transpose(1, 0, 2))
    xT16 = xT32.astype(NP_BF16)
    gwT = np.ascontiguousarray(gate_w.T.reshape(KO, P, E).transpose(1, 0, 2)).astype(
        np.float32
    )
    gb_b = np.broadcast_to(gate_bias[None, :], (P, E)).astype(np.float32).copy()

    in_maps = []
    for c in range(N_CORES):
        # routed expert weights: W1[c] [2I, D] -> interleaved v/g pair tiles
        A = W1[c].reshape(2, JR, P, KO, P)  # (vg, j, m, ko, p)
        w1t = np.ascontiguousarray(
            A.transpose(4, 1, 3, 0, 2).reshape(P, JR, KO, 2 * P)
        ).astype(NP_BF16)
        b1t = np.ascontiguousarray(
            b1[c].reshape(2, JR, P).transpose(2, 1, 0).reshape(P, 2 * JR)
        ).astype(np.float32)

        # shared expert slice (padded to JS pair tiles)
        sl = _shared_slices(c)
        A_sh = np.zeros((2, JS, P, D), np.float32)
        bs1t_raw = np.zeros((2, JS, P), np.float32)
        Wd_sh = np.zeros((JS, P, D), np.float32)
        for jj, jglob in enumerate(sl):
            rows = slice(jglob * P, (jglob + 1) * P)
            A_sh[0, jj] = Ws1[rows.start : rows.stop]
            A_sh[1, jj] = Ws1[I_SH + rows.start : I_SH + rows.stop]
            bs1t_raw[0, jj] = bs1[rows]
            bs1t_raw[1, jj] = bs1[I_SH + rows.start : I_SH + rows.stop]
            Wd_sh[jj] = Ws2[:, rows].T
        ws1t = np.ascontiguousarray(
            A_sh.reshape(2, JS, P, KO, P).transpose(4, 1, 3, 0, 2).reshape(
                P, JS, KO, 2 * P
            )
        ).astype(NP_BF16)
        bs1t = np.ascontiguousarray(
            bs1t_raw.transpose(2, 1, 0).reshape(P, 2 * JS)
        ).astype(np.float32)

        # down weights: [W2[c].T ; shared slices] -> [128, KD, DT, 128]
        Wd = np.concatenate([W2[c].T, Wd_sh.reshape(JS * P, D)], axis=0)
        wd = np.ascontiguousarray(
            Wd.reshape(KD, P, DT, P).transpose(1, 0, 2, 3)
        ).astype(NP_BF16)

        sel_b = np.zeros((P, E), np.float32)
        sel_b[:, c] = 1.0
        bs2_c = bs2 if c == 0 else np.zeros_like(bs2)

        in_maps.append(
            {
                "xT": xT16,
                "xT32": xT32,
                "gwT": gwT,
                "gbias": gb_b,
                "sel": sel_b,
                "w1t": w1t,
                "ws1t": ws1t,
                "wd": wd,
                "b1t": b1t,
                "bs1t": bs1t,
                "b2c": _part_tiles(b2[c], DT),
                "bs2c": _part_tiles(bs2_c, DT),
            }
        )
    return in_maps


_NC_CACHE = {}


def get_nc():
    if "nc" not in _NC_CACHE:
        _NC_CACHE["nc"] = build_nc()
    return _NC_CACHE["nc"]


def combine_outputs(results):
    """Per-core result dicts -> full [B, S, D] float32 output."""
    acc = np.zeros((D, T), np.float64)
    for r in results:
        acc += np.asarray(r["out"], np.float32)
    return np.ascontiguousarray(acc.T.reshape(B, S, D).astype(np.float32))


def kernel(**inputs):
    nc = get_nc()
    in_maps = prep_inputs(inputs)
    res = run_bass_kernel_spmd(nc, in_maps, core_ids=list(range(N_CORES)))
    return combine_outputs(res.results)


if __name__ == "__main__":
    # quick self-drive (requires reference.py next to this file)
    import reference

    inputs = {k: np.asarray(v) for k, v in reference.setup_inputs().items()}
    out = kernel(**inputs)
    exp = np.asarray(reference.reference(**inputs))
    err = np.abs(out - exp).max()
    rel = np.abs(out - exp).max() / np.abs(exp).max()
    print("absmax err:", err, "rel:", rel)



# revision 3
# speedup vs baseline: 3.2809x; 3.2809x over previous
"""Kimi-style MoE (8 routed experts top-2 + shared expert) on 8 Trainium2 cores.

Strategy: token-level expert routing is computed on the host (gate + top-k +
gather in prep; scatter/combine after), so the device kernel is pure dense
swiglu-MLP work on pre-gathered tokens. Each core runs two fixed-shape
"segments" of identical structure (up-proj [2816,1024] -> swiglu -> down-proj
[1024,1408]):

  segment A (size SA): the core's routed expert applied to that expert's
      gathered tokens (padded with zeros to SA >= max expert token count).
  segment B (size SB=T/4): one *half* of the shared expert's intermediate
      (1408 of 2816 channels -- exactly the shape of one routed expert)
      applied to a contiguous quarter of all tokens. Core c takes shared
      half c//4 and token range c%4; the two halves' partials sum on host.

This computes only the top-2-of-8 routed work (vs dense-over-E), cutting
per-core PE work from ~11.1e9 to ~5e9 MACs. All matmuls run in bf16 with
fp32 PSUM accumulation. Per-core outputs are raw segment outputs [D, S];
the host applies gate weights, down-proj biases, and the scatter-add.
"""

import sys

for _p in ("/opt/trn_rl_repo", "/opt/pypackages"):
    if _p not in sys.path:
        sys.path.insert(0, _p)

import numpy as np
import ml_dtypes

import concourse.bass as bass
import concourse.mybir as mybir
import concourse.tile as tile
from concourse import bacc
from concourse.bass import ts
from concourse.bass_utils import run_bass_kernel_spmd

BF16 = mybir.dt.bfloat16
F32 = mybir.dt.float32
NP_BF16 = ml_dtypes.bfloat16

# Problem shapes (hardcoded per the contract).
B, S, D = 2, 1024, 1024
E, TOPK = 8, 2
I = 1408
N_SHARED = 2
I_SH = N_SHARED * I          # 2816
SCALE = 2.5
T = B * S                    # 2048
P = 128
KO = D // P                  # 8 contraction subtiles
JR = I // P                  # 11 (v,g) pair tiles per segment
DT = D // P                  # 8 output partition tiles
N_CORES = 8
SB = T // 4                  # 512 tokens per shared-half segment


def _chunks(S_seg):
    """Split a segment's token dim into PSUM-sized (<=512) chunks."""
    n = -(-S_seg // 512)
    base = -(-S_seg // (16 * n)) * 16
    out, c0 = [], 0
    while c0 < S_seg:
        cn = min(base, S_seg - c0)
        out.append((c0, cn))
        c0 += cn
    return out


def _body(tc, io, SA):
    nc = tc.nc
    add = mybir.AluOpType.add
    mult = mybir.AluOpType.mult
    segs = [("a", SA), ("b", SB)]

    with (
        tc.tile_pool(name="const", bufs=1) as cpool,
        tc.tile_pool(name="wup", bufs=4) as wpool,
        tc.tile_pool(name="sv", bufs=4) as svpool,
        tc.tile_pool(name="outs", bufs=4) as opool,
    ):
        xs, hs, bups, wdns = {}, {}, {}, {}
        for s, S_seg in segs:
            xs[s] = cpool.tile([P, KO, S_seg], BF16, tag=f"x_{s}", name=f"x_{s}")
            hs[s] = cpool.tile([P, JR, S_seg], BF16, tag=f"h_{s}", name=f"h_{s}")
            bups[s] = cpool.tile([P, 2 * JR], F32, tag=f"bup_{s}", name=f"bup_{s}")
            wdns[s] = cpool.tile(
                [P, JR, DT, P], BF16, tag=f"wdn_{s}", name=f"wdn_{s}"
            )
            nc.sync.dma_start(xs[s][:], io[f"x_{s}"][:])
            nc.sync.dma_start(bups[s][:], io[f"bup_{s}"][:])

        # ---- up projections + swiglu -> h (segments interleaved per j) ----
        with tc.tile_pool(name="upsum", bufs=4, space="PSUM") as upsum:
            for j in range(JR):
                for s, S_seg in segs:
                    wtile = wpool.tile([P, KO, 2 * P], BF16, tag=f"w_{s}")
                    nc.sync.dma_start(wtile[:], io[f"wup_{s}"][:, j])
                    bias_v = bups[s][:, 2 * j : 2 * j + 1]
                    bias_g = bups[s][:, 2 * j + 1 : 2 * j + 2]
                    for c0, cn in _chunks(S_seg):
                        pv = upsum.tile([P, 512], F32, tag="pv")
                        pg = upsum.tile([P, 512], F32, tag="pg")
                        for k in range(KO):
                            nc.tensor.matmul(
                                pv[:, :cn], wtile[:, k, :P],
                                xs[s][:, k, c0 : c0 + cn],
                                start=(k == 0), stop=(k == KO - 1),
                            )
                        for k in range(KO):
                            nc.tensor.matmul(
                                pg[:, :cn], wtile[:, k, P:],
                                xs[s][:, k, c0 : c0 + cn],
                                start=(k == 0), stop=(k == KO - 1),
                            )
                        sv = svpool.tile([P, 512], F32, tag="sv")
                        # silu built from sigmoid to match jax numerics:
                        # sv = (v+bv) * sigmoid(v+bv); h = (g+bg) * sv
                        nc.scalar.activation(
                            sv[:, :cn], pv[:, :cn],
                            mybir.ActivationFunctionType.Sigmoid, bias=bias_v,
                        )
                        nc.vector.scalar_tensor_tensor(
                            sv[:, :cn], pv[:, :cn], bias_v, sv[:, :cn], add, mult
                        )
                        nc.vector.scalar_tensor_tensor(
                            hs[s][:, j, c0 : c0 + cn], pg[:, :cn], bias_g,
                            sv[:, :cn], add, mult,
                        )

        # down-proj weights stream in during the tail of the up phase
        for s, _ in segs:
            nc.sync.dma_start(wdns[s][:], io[f"wdn_{s}"][:])

        # ---- down projection -> out (raw, biases applied on host) ----
        with tc.tile_pool(name="dpsum", bufs=4, space="PSUM") as dpsum:
            for dt in range(DT):
                for s, S_seg in segs:
                    for c0, cn in _chunks(S_seg):
                        pd = dpsum.tile([P, 512], F32, tag="pd")
                        for kd in range(JR):
                            nc.tensor.matmul(
                                pd[:, :cn], wdns[s][:, kd, dt],
                                hs[s][:, kd, c0 : c0 + cn],
                                start=(kd == 0), stop=(kd == JR - 1),
                            )
                        osb = opool.tile([P, 512], F32, tag="osb")
                        nc.vector.tensor_copy(osb[:, :cn], pd[:, :cn])
                        nc.sync.dma_start(
                            io[f"out_{s}"][ts(dt, P), c0 : c0 + cn], osb[:, :cn]
                        )


def build_nc(SA, reps=1):
    nc = bacc.Bacc(None, target_bir_lowering=False, debug=False)
    io = {}
    for s, S_seg in (("a", SA), ("b", SB)):
        io[f"x_{s}"] = nc.declare_dram_parameter(
            f"x_{s}", [P, KO, S_seg], BF16, isOutput=False)
        io[f"wup_{s}"] = nc.declare_dram_parameter(
            f"wup_{s}", [P, JR, KO, 2 * P], BF16, isOutput=False)
        io[f"wdn_{s}"] = nc.declare_dram_parameter(
            f"wdn_{s}", [P, JR, DT, P], BF16, isOutput=False)
        io[f"bup_{s}"] = nc.declare_dram_parameter(
            f"bup_{s}", [P, 2 * JR], F32, isOutput=False)
        io[f"out_{s}"] = nc.declare_dram_parameter(
            f"out_{s}", [D, S_seg], F32, isOutput=True)
    with tile.TileContext(nc) as tc:
        for _ in range(reps):
            _body(tc, io, SA)
    nc.compile()
    return nc


# ---------------- host-side routing / pack / combine ----------------

def route(x, gate_w, gate_bias):
    """Gate on host: topk_idx [T,K], topk_w [T,K] (renormalized * SCALE)."""
    xf = x.reshape(T, D).astype(np.float32)
    logits = xf @ gate_w.T.astype(np.float32)
    scores = 1.0 / (1.0 + np.exp(-logits))
    sfc = scores + gate_bias[None, :].astype(np.float32)
    topk_idx = np.argsort(-sfc, axis=-1, kind="stable")[:, :TOPK]
    topk_w = np.take_along_axis(sfc, topk_idx, axis=-1)
    topk_w = topk_w / (topk_w.sum(-1, keepdims=True) + 1e-20) * SCALE
    return topk_idx, topk_w.astype(np.float32)


def make_plan(inputs):
    """Routing plan: per-expert token lists, inverse positions, SA."""
    topk_idx, topk_w = route(inputs["x"], inputs["gate_w"], inputs["gate_bias"])
    flat_e = topk_idx.reshape(-1)
    order = np.argsort(flat_e, kind="stable")
    counts = np.bincount(flat_e, minlength=E)
    starts = np.zeros(E + 1, np.int64)
    starts[1:] = np.cumsum(counts)
    tok_of = order // TOPK
    pos = np.empty(T * TOPK, np.int64)
    pos[order] = np.arange(T * TOPK) - starts[flat_e[order]]
    tok_lists = [tok_of[starts[e] : starts[e + 1]] for e in range(E)]
    SA = max(512, -(-int(counts.max()) // 16) * 16)
    return {
        "topk_idx": topk_idx,
        "topk_w": topk_w,
        "tok_lists": tok_lists,
        "pos": pos.reshape(T, TOPK),
        "SA": SA,
    }


def _up_layout(Wv, Wg):
    """[I,D]x2 -> [P, JR, KO, 2P] bf16 (v/g interleaved pair tiles)."""
    A = np.stack([Wv, Wg], 0).reshape(2, JR, P, KO, P)
    return np.ascontiguousarray(
        A.transpose(4, 1, 3, 0, 2).reshape(P, JR, KO, 2 * P)
    ).astype(NP_BF16)


def _dn_layout(WdT):
    """[I, D] (already transposed W2.T) -> [P, JR, DT, P] bf16."""
    return np.ascontiguousarray(
        WdT.reshape(JR, P, DT, P).transpose(1, 0, 2, 3)
    ).astype(NP_BF16)


def _bup_layout(bv, bg):
    """[I]x2 -> [P, 2*JR] f32 (v/g interleaved per pair tile)."""
    A = np.stack([bv, bg], 0).reshape(2, JR, P)
    return np.ascontiguousarray(A.transpose(2, 1, 0).reshape(P, 2 * JR)).astype(
        np.float32
    )


def _x_layout(xrows, S_seg):
    """[n, D] f32 tokens -> [P, KO, S_seg] bf16 (d on partitions), zero pad."""
    xp = np.zeros((S_seg, D), np.float32)
    xp[: len(xrows)] = xrows
    return np.ascontiguousarray(
        xp.T.reshape(KO, P, S_seg).transpose(1, 0, 2)
    ).astype(NP_BF16)


def prep_inputs(inputs, plan):
    """Full problem inputs + plan -> list of 8 per-core in_maps."""
    xf = np.asarray(inputs["x"], np.float32).reshape(T, D)
    W1 = np.asarray(inputs["W1"], np.float32)
    b1 = np.asarray(inputs["b1"], np.float32)
    W2 = np.asarray(inputs["W2"], np.float32)
    Ws1 = np.asarray(inputs["Ws1"], np.float32)
    bs1 = np.asarray(inputs["bs1"], np.float32)
    Ws2 = np.asarray(inputs["Ws2"], np.float32)
    SA = plan["SA"]

    in_maps = []
    for c in range(N_CORES):
        h, r = c // 4, c % 4
        hsl = slice(h * I, (h + 1) * I)
        in_maps.append(
            {
                "x_a": _x_layout(xf[plan["tok_lists"][c]], SA),
                "x_b": _x_layout(xf[r * SB : (r + 1) * SB], SB),
                "wup_a": _up_layout(W1[c, :I], W1[c, I:]),
                "wup_b": _up_layout(Ws1[hsl], Ws1[I_SH + h * I : I_SH + (h + 1) * I]),
                "wdn_a": _dn_layout(W2[c].T),
                "wdn_b": _dn_layout(Ws2[:, hsl].T),
                "bup_a": _bup_layout(b1[c, :I], b1[c, I:]),
                "bup_b": _bup_layout(bs1[hsl], bs1[I_SH + h * I : I_SH + (h + 1) * I]),
            }
        )
    return in_maps


def combine_outputs(results, inputs, plan):
    """Per-core raw segment outputs -> full [B, S, D] float32 output."""
    b2 = np.asarray(inputs["b2"], np.float32)
    bs2 = np.asarray(inputs["bs2"], np.float32)
    SA = plan["SA"]
    out = np.zeros((T, D), np.float32)
    # shared halves: contiguous token quarters, two partials each
    for c in range(N_CORES):
        r = c % 4
        out[r * SB : (r + 1) * SB] += np.asarray(results[c]["out_b"], np.float32).T
    out += bs2[None, :]
    # routed: gather-form combine (y already excludes b2; add w*(y + b2[e]))
    y_flat = np.stack(
        [np.asarray(results[c]["out_a"], np.float32).T for c in range(N_CORES)]
    ).reshape(N_CORES * SA, D)
    topk_idx, topk_w, pos = plan["topk_idx"], plan["topk_w"], plan["pos"]
    for k in range(TOPK):
        e_k = topk_idx[:, k]
        out += topk_w[:, k : k + 1] * (y_flat[e_k * SA + pos[:, k]] + b2[e_k])
    return np.ascontiguousarray(out.reshape(B, S, D))


_NC_CACHE = {}


def get_nc(SA, reps=1):
    key = (SA, reps)
    if key not in _NC_CACHE:
        _NC_CACHE[key] = build_nc(SA, reps=reps)
    return _NC_CACHE[key]


def kernel(**inputs):
    plan = make_plan(inputs)
    nc = get_nc(plan["SA"])
    in_maps = prep_inputs(inputs, plan)
    res = run_bass_kernel_spmd(nc, in_maps, core_ids=list(range(N_CORES)))
    return combine_outputs(res.results, inputs, plan)


if __name__ == "__main__":
    # quick self-drive (requires reference.py next to this file)
    import reference

    inputs = {k: np.asarray(v) for k, v in reference.setup_inputs().items()}
    out = kernel(**inputs)
    exp = np.asarray(reference.reference(**inputs))
    err = np.abs(out - exp).max()
    rel = err / np.abs(exp).max()
    print("absmax err:", err, "rel:", rel)


# revision 9
# speedup vs baseline: 4.0153x; 1.2238x over previous
"""Kimi-style MoE (8 routed experts top-2 + shared expert) on 8 Trainium2 cores.

Strategy: token-level expert routing is computed on the host (gate + top-k +
gather in prep; scatter/combine after), so the device kernel is pure dense
swiglu-MLP work on pre-gathered tokens. Each core runs two fixed-shape
"segments" of identical structure (up-proj [2816,1024] -> swiglu -> down-proj
[1024,1408]):

  segment A (size SA): the core's routed expert applied to that expert's
      gathered tokens (padded with zeros to SA >= max expert token count).
  segment B (size SB=T/4): one *half* of the shared expert's intermediate
      (1408 of 2816 channels -- exactly the shape of one routed expert)
      applied to a contiguous quarter of all tokens. Core c takes shared
      half c//4 and token range c%4; the two halves' partials sum on host.

This computes only the top-2-of-8 routed work (vs dense-over-E), cutting
per-core PE work from ~11.1e9 to ~5e9 MACs. All matmuls run in bf16 with
fp32 PSUM accumulation. Per-core outputs are raw segment outputs [D, S];
the host applies gate weights, down-proj biases, and the scatter-add.
"""

import sys

for _p in ("/opt/trn_rl_repo", "/opt/pypackages"):
    if _p not in sys.path:
        sys.path.insert(0, _p)

import numpy as np
import ml_dtypes

import concourse.bass as bass
import concourse.mybir as mybir
import concourse.tile as tile
from concourse import bacc
from concourse.bass import ts
from concourse.bass_utils import run_bass_kernel_spmd

BF16 = mybir.dt.bfloat16
F32 = mybir.dt.float32
NP_BF16 = ml_dtypes.bfloat16

# Problem shapes (hardcoded per the contract).
B, S, D = 2, 1024, 1024
E, TOPK = 8, 2
I = 1408
N_SHARED = 2
I_SH = N_SHARED * I          # 2816
SCALE = 2.5
T = B * S                    # 2048
P = 128
KO = D // P                  # 8 contraction subtiles
JR = I // P                  # 11 (v,g) pair tiles per segment
DT = D // P                  # 8 output partition tiles
N_CORES = 8
SB = T // 4                  # 512 tokens per shared-half segment


def _chunks(S_seg):
    """Split a segment's token dim into PSUM-sized (<=512) chunks."""
    n = -(-S_seg // 512)
    base = -(-S_seg // (16 * n)) * 16
    out, c0 = [], 0
    while c0 < S_seg:
        cn = min(base, S_seg - c0)
        out.append((c0, cn))
        c0 += cn
    return out


def _body(tc, io, SA, SC):
    nc = tc.nc
    add = mybir.AluOpType.add
    mult = mybir.AluOpType.mult
    segs = [("a", SA), ("b", SB)] + ([("c", SC)] if SC else [])

    with (
        tc.tile_pool(name="const", bufs=1) as cpool,
        tc.tile_pool(name="wup", bufs=4) as wpool,
        tc.tile_pool(name="sv", bufs=4) as svpool,
        tc.tile_pool(name="outs", bufs=4) as opool,
    ):
        xs, hs, bups, wdns = {}, {}, {}, {}
        for s, S_seg in segs:
            xs[s] = cpool.tile([P, KO, S_seg], BF16, tag=f"x_{s}", name=f"x_{s}")
            hs[s] = cpool.tile([P, JR, S_seg], BF16, tag=f"h_{s}", name=f"h_{s}")
            bups[s] = cpool.tile([P, 2 * JR], F32, tag=f"bup_{s}", name=f"bup_{s}")
            wdns[s] = cpool.tile(
                [P, JR, DT, P], BF16, tag=f"wdn_{s}", name=f"wdn_{s}"
            )
            nc.sync.dma_start(xs[s][:], io[f"x_{s}"][:])
            nc.sync.dma_start(bups[s][:], io[f"bup_{s}"][:])

        # ---- up projections + swiglu -> h (segments interleaved per j) ----
        with tc.tile_pool(name="upsum", bufs=4, space="PSUM") as upsum:
            for j in range(JR):
                for s, S_seg in segs:
                    wtile = wpool.tile([P, KO, 2 * P], BF16, tag=f"w_{s}")
                    nc.sync.dma_start(wtile[:], io[f"wup_{s}"][:, j])
                    bias_v = bups[s][:, 2 * j : 2 * j + 1]
                    bias_g = bups[s][:, 2 * j + 1 : 2 * j + 2]
                    for c0, cn in _chunks(S_seg):
                        pv = upsum.tile([P, 512], F32, tag="pv")
                        pg = upsum.tile([P, 512], F32, tag="pg")
                        for k in range(KO):
                            nc.tensor.matmul(
                                pv[:, :cn], wtile[:, k, :P],
                                xs[s][:, k, c0 : c0 + cn],
                                start=(k == 0), stop=(k == KO - 1),
                            )
                        for k in range(KO):
                            nc.tensor.matmul(
                                pg[:, :cn], wtile[:, k, P:],
                                xs[s][:, k, c0 : c0 + cn],
                                start=(k == 0), stop=(k == KO - 1),
                            )
                        sv = svpool.tile([P, 512], F32, tag="sv")
                        # silu built from sigmoid to match jax numerics:
                        # sv = (v+bv) * sigmoid(v+bv); h = (g+bg) * sv
                        nc.scalar.activation(
                            sv[:, :cn], pv[:, :cn],
                            mybir.ActivationFunctionType.Sigmoid, bias=bias_v,
                        )
                        nc.vector.scalar_tensor_tensor(
                            sv[:, :cn], pv[:, :cn], bias_v, sv[:, :cn], add, mult
                        )
                        nc.vector.scalar_tensor_tensor(
                            hs[s][:, j, c0 : c0 + cn], pg[:, :cn], bias_g,
                            sv[:, :cn], add, mult,
                        )

        # down-proj weights stream in during the tail of the up phase
        for s, _ in segs:
            nc.sync.dma_start(wdns[s][:], io[f"wdn_{s}"][:])

        # ---- down projection -> out (raw, biases applied on host) ----
        with tc.tile_pool(name="dpsum", bufs=4, space="PSUM") as dpsum:
            for dt in range(DT):
                for s, S_seg in segs:
                    for c0, cn in _chunks(S_seg):
                        pd = dpsum.tile([P, 512], F32, tag="pd")
                        for kd in range(JR):
                            nc.tensor.matmul(
                                pd[:, :cn], wdns[s][:, kd, dt],
                                hs[s][:, kd, c0 : c0 + cn],
                                start=(kd == 0), stop=(kd == JR - 1),
                            )
                        osb = opool.tile([P, 512], F32, tag="osb")
                        nc.vector.tensor_copy(osb[:, :cn], pd[:, :cn])
                        nc.sync.dma_start(
                            io[f"out_{s}"][ts(dt, P), c0 : c0 + cn], osb[:, :cn]
                        )


def build_nc(SA, SC, reps=1):
    nc = bacc.Bacc(None, target_bir_lowering=False, debug=False)
    io = {}
    segs = [("a", SA), ("b", SB)] + ([("c", SC)] if SC else [])
    for s, S_seg in segs:
        io[f"x_{s}"] = nc.declare_dram_parameter(
            f"x_{s}", [P, KO, S_seg], BF16, isOutput=False)
        io[f"wup_{s}"] = nc.declare_dram_parameter(
            f"wup_{s}", [P, JR, KO, 2 * P], BF16, isOutput=False)
        io[f"wdn_{s}"] = nc.declare_dram_parameter(
            f"wdn_{s}", [P, JR, DT, P], BF16, isOutput=False)
        io[f"bup_{s}"] = nc.declare_dram_parameter(
            f"bup_{s}", [P, 2 * JR], F32, isOutput=False)
        io[f"out_{s}"] = nc.declare_dram_parameter(
            f"out_{s}", [D, S_seg], F32, isOutput=True)
    with tile.TileContext(nc) as tc:
        for _ in range(reps):
            _body(tc, io, SA, SC)
    nc.compile()
    return nc


# ---------------- host-side routing / pack / combine ----------------

def route(x, gate_w, gate_bias):
    """Gate on host: topk_idx [T,K], topk_w [T,K] (renormalized * SCALE)."""
    xf = x.reshape(T, D).astype(np.float32)
    logits = xf @ gate_w.T.astype(np.float32)
    scores = 1.0 / (1.0 + np.exp(-logits))
    sfc = scores + gate_bias[None, :].astype(np.float32)
    topk_idx = np.argsort(-sfc, axis=-1, kind="stable")[:, :TOPK]
    topk_w = np.take_along_axis(sfc, topk_idx, axis=-1)
    topk_w = topk_w / (topk_w.sum(-1, keepdims=True) + 1e-20) * SCALE
    return topk_idx, topk_w.astype(np.float32)


def _pack_segments(counts):
    """Pick (SA, SC): every expert's first SA tokens go to its own core's
    A-segment; overflow spills to per-core C-segments (size SC, one expert
    per C-seg, <= N_CORES of them). Minimizes PE cost (token-units, with a
    small-matmul efficiency penalty on SC)."""
    max_c = int(counts.max())
    best = None
    for SA in range(512, max(512, -(-max_c // 16) * 16) + 16, 16):
        over = [max(0, int(n) - SA) for n in counts]
        tot_over = sum(over)
        if tot_over == 0:
            cand = (float(SA), SA, 0)
        else:
            cand = None
            for SC in range(16, 513, 16):
                nsegs = sum(-(-o // SC) for o in over if o)
                if nsegs <= N_CORES:
                    eff = max(SC + 4, 68) / SC  # small-N matmul overhead
                    c = (SA + SC * eff, SA, SC)
                    if cand is None or c[0] < cand[0]:
                        cand = c
            if cand is None:
                continue
        if best is None or cand[0] < best[0]:
            best = cand
    return best[1], best[2]


def make_plan(inputs):
    """Routing plan: per-expert token lists, inverse positions, segment sizes,
    and the C-segment (overflow spill) assignment table."""
    topk_idx, topk_w = route(inputs["x"], inputs["gate_w"], inputs["gate_bias"])
    flat_e = topk_idx.reshape(-1)
    order = np.argsort(flat_e, kind="stable")
    counts = np.bincount(flat_e, minlength=E)
    starts = np.zeros(E + 1, np.int64)
    starts[1:] = np.cumsum(counts)
    tok_of = order // TOPK
    pos = np.empty(T * TOPK, np.int64)
    pos[order] = np.arange(T * TOPK) - starts[flat_e[order]]
    tok_lists = [tok_of[starts[e] : starts[e + 1]] for e in range(E)]
    SA, SC = _pack_segments(counts)
    # C-seg assignment: cseg_expert[c] = expert whose overflow chunk lives on
    # core c (or -1); cseg_core[e, j] = core holding the j-th overflow chunk.
    cseg_expert = np.full(N_CORES, -1, np.int64)
    cseg_off = np.zeros(N_CORES, np.int64)
    max_j = 1 if SC == 0 else max(1, -(-max(0, int(counts.max()) - SA) // max(SC, 1)))
    cseg_core = np.full((E, max_j), -1, np.int64)
    if SC:
        core = 0
        for e in range(E):
            o = max(0, int(counts[e]) - SA)
            j = 0
            while o > 0:
                cseg_expert[core] = e
                cseg_off[core] = SA + j * SC
                cseg_core[e, j] = core
                core += 1
                j += 1
                o -= SC
    return {
        "topk_idx": topk_idx,
        "topk_w": topk_w,
        "tok_lists": tok_lists,
        "pos": pos.reshape(T, TOPK),
        "SA": SA,
        "SC": SC,
        "cseg_expert": cseg_expert,
        "cseg_off": cseg_off,
        "cseg_core": cseg_core,
    }


def _up_layout(Wv, Wg):
    """[I,D]x2 -> [P, JR, KO, 2P] bf16 (v/g interleaved pair tiles)."""
    A = np.stack([Wv, Wg], 0).reshape(2, JR, P, KO, P)
    return np.ascontiguousarray(
        A.transpose(4, 1, 3, 0, 2).reshape(P, JR, KO, 2 * P)
    ).astype(NP_BF16)


def _dn_layout(WdT):
    """[I, D] (already transposed W2.T) -> [P, JR, DT, P] bf16."""
    return np.ascontiguousarray(
        WdT.reshape(JR, P, DT, P).transpose(1, 0, 2, 3)
    ).astype(NP_BF16)


def _bup_layout(bv, bg):
    """[I]x2 -> [P, 2*JR] f32 (v/g interleaved per pair tile)."""
    A = np.stack([bv, bg], 0).reshape(2, JR, P)
    return np.ascontiguousarray(A.transpose(2, 1, 0).reshape(P, 2 * JR)).astype(
        np.float32
    )


def _x_layout(xrows, S_seg):
    """[n, D] f32 tokens -> [P, KO, S_seg] bf16 (d on partitions), zero pad."""
    xp = np.zeros((S_seg, D), np.float32)
    xp[: len(xrows)] = xrows
    return np.ascontiguousarray(
        xp.T.reshape(KO, P, S_seg).transpose(1, 0, 2)
    ).astype(NP_BF16)


def prep_inputs(inputs, plan):
    """Full problem inputs + plan -> list of 8 per-core in_maps."""
    xf = np.asarray(inputs["x"], np.float32).reshape(T, D)
    W1 = np.asarray(inputs["W1"], np.float32)
    b1 = np.asarray(inputs["b1"], np.float32)
    W2 = np.asarray(inputs["W2"], np.float32)
    Ws1 = np.asarray(inputs["Ws1"], np.float32)
    bs1 = np.asarray(inputs["bs1"], np.float32)
    Ws2 = np.asarray(inputs["Ws2"], np.float32)
    SA, SC = plan["SA"], plan["SC"]

    in_maps = []
    for c in range(N_CORES):
        h, r = c // 4, c % 4
        hsl = slice(h * I, (h + 1) * I)
        m = {
            "x_a": _x_layout(xf[plan["tok_lists"][c][:SA]], SA),
            "x_b": _x_layout(xf[r * SB : (r + 1) * SB], SB),
            "wup_a": _up_layout(W1[c, :I], W1[c, I:]),
            "wup_b": _up_layout(Ws1[hsl], Ws1[I_SH + h * I : I_SH + (h + 1) * I]),
            "wdn_a": _dn_layout(W2[c].T),
            "wdn_b": _dn_layout(Ws2[:, hsl].T),
            "bup_a": _bup_layout(b1[c, :I], b1[c, I:]),
            "bup_b": _bup_layout(bs1[hsl], bs1[I_SH + h * I : I_SH + (h + 1) * I]),
        }
        if SC:
            e = int(plan["cseg_expert"][c])
            if e >= 0:
                off = int(plan["cseg_off"][c])
                m["x_c"] = _x_layout(xf[plan["tok_lists"][e][off : off + SC]], SC)
                m["wup_c"] = _up_layout(W1[e, :I], W1[e, I:])
                m["wdn_c"] = _dn_layout(W2[e].T)
                m["bup_c"] = _bup_layout(b1[e, :I], b1[e, I:])
            else:
                m["x_c"] = np.zeros((P, KO, SC), NP_BF16)
                m["wup_c"] = np.zeros((P, JR, KO, 2 * P), NP_BF16)
                m["wdn_c"] = np.zeros((P, JR, DT, P), NP_BF16)
                m["bup_c"] = np.zeros((P, 2 * JR), np.float32)
        in_maps.append(m)
    return in_maps


def combine_outputs(results, inputs, plan):
    """Per-core raw segment outputs -> full [B, S, D] float32 output."""
    b2 = np.asarray(inputs["b2"], np.float32)
    bs2 = np.asarray(inputs["bs2"], np.float32)
    SA, SC = plan["SA"], plan["SC"]
    out = np.zeros((T, D), np.float32)
    # shared halves: contiguous token quarters, two partials each
    for c in range(N_CORES):
        r = c % 4
        out[r * SB : (r + 1) * SB] += np.asarray(results[c]["out_b"], np.float32).T
    out += bs2[None, :]
    # routed: gather-form combine (y already excludes b2; add w*(y + b2[e]))
    # flat layout per core: [out_a (SA) | out_c (SC)]
    stride = SA + SC
    pieces = []
    for c in range(N_CORES):
        pieces.append(np.asarray(results[c]["out_a"], np.float32).T)
        if SC:
            pieces.append(np.asarray(results[c]["out_c"], np.float32).T)
    y_flat = np.concatenate(pieces, axis=0)
    topk_idx, topk_w, pos = plan["topk_idx"], plan["topk_w"], plan["pos"]
    cseg_core = plan["cseg_core"]
    for k in range(TOPK):
        e_k = topk_idx[:, k]
        p = pos[:, k]
        flat = e_k * stride + p
        if SC:
            ov = p >= SA
            if ov.any():
                q = p[ov] - SA
                core = cseg_core[e_k[ov], q // SC]
                flat[ov] = core * stride + SA + q % SC
        out += topk_w[:, k : k + 1] * (y_flat[flat] + b2[e_k])
    return np.ascontiguousarray(out.reshape(B, S, D))


_NC_CACHE = {}


def get_nc(SA, SC, reps=1):
    key = (SA, SC, reps)
    if key not in _NC_CACHE:
        _NC_CACHE[key] = build_nc(SA, SC, reps=reps)
    return _NC_CACHE[key]


def kernel(**inputs):
    plan = make_plan(inputs)
    nc = get_nc(plan["SA"], plan["SC"])
    in_maps = prep_inputs(inputs, plan)
    res = run_bass_kernel_spmd(nc, in_maps, core_ids=list(range(N_CORES)))
    return combine_outputs(res.results, inputs, plan)


if __name__ == "__main__":
    # quick self-drive (requires reference.py next to this file)
    import reference

    inputs = {k: np.asarray(v) for k, v in reference.setup_inputs().items()}
    out = kernel(**inputs)
    exp = np.asarray(reference.reference(**inputs))
    err = np.abs(out - exp).max()
    rel = err / np.abs(exp).max()
    print("absmax err:", err, "rel:", rel)
